# revision 11
# baseline (speedup 1.0000x reference)
"""AttnDecoderRNN single-step on 8 axon-tunneled TRN2 NeuronCores.

Strategy (tensor parallel over 8 cores, batch=1):
  - embedding: host gathers the single row emb[input] (only 8KB read).
  - attention (attn_W [512,4096], encoder_outputs [512,2048]): replicated on
    every core (small weights; avoids two collectives).
  - comb_W [2048,4096]: row-sharded (core k computes x_k = x[256k:256k+256]).
  - GRU W_ih/W_hh [6144,2048]: column-sharded with slices matched to x_k /
    h0_k, producing partial pre-gates; one AllReduce(48KB) sums them, then
    every core computes the full h1 redundantly (cheap elementwise).
  - out_W [50257,2048]: vocab-sharded (6283 rows/core, padded), streamed from
    HBM in a flat per-partition-contiguous layout; logits z_k per core with
    per-tile max / sum-exp stats; tail AllGather(16B) of (max, sumexp) gives
    the global log-normalizer; each core emits its log_prob shard.
  - host concatenates shards and trims padding.

All activation vectors live in "column layout" [128, C] (element i at
partition i%128, column i//128) so they feed matmul lhsT chunks [128,1]
directly. PE matmuls run fp32 (exact 2-pass mode) for the serial chain and
float32r (1 cyc/col) for the big streamed output projection by default.
"""

import os
import sys

import numpy as np

H = 2048
L = 512
V = 50257
N_CORES = 8
VS = 6283            # vocab rows per core (8*6283 = 50264 >= 50257)
NT = 12              # full 512-wide output tiles per core
TAIL = VS - NT * 512  # 139
NKH = H // 128       # 16 chunks over H
NEG_BIG = -1.0e30

F32R_OUT = os.environ.get("BASS_F32R_OUT", "1") == "1"
F32R_CHAIN = os.environ.get("BASS_F32R_CHAIN", "0") == "1"

_CACHE = {}


def _build():
    sys.path.insert(0, "/opt/trn_rl_repo")
    import concourse.bass as bass  # noqa: F401
    import concourse.tile as tile
    from concourse import bacc, mybir

    f32 = mybir.dt.float32
    f32r = mybir.dt.float32r
    AF = mybir.ActivationFunctionType
    AX = mybir.AxisListType
    ALU = mybir.AluOpType
    RG = [list(range(N_CORES))]

    wdt_out = f32r if F32R_OUT else f32
    wdt_ch = f32r if F32R_CHAIN else f32

    nc = bacc.Bacc("TRN2", target_bir_lowering=False, debug=False,
                   num_devices=N_CORES)

    # ---- I/O ----
    d_emb_col = nc.dram_tensor("emb_col", [128, NKH], f32, kind="ExternalInput")
    d_h0_col = nc.dram_tensor("h0_col", [128, NKH], f32, kind="ExternalInput")
    d_h0sl_col = nc.dram_tensor("h0sl_col", [128, 2], f32, kind="ExternalInput")
    d_attn_w = nc.dram_tensor("attn_w", [128, 32 * 512], f32, kind="ExternalInput")
    d_attn_b = nc.dram_tensor("attn_b", [1, 512], f32, kind="ExternalInput")
    d_enc = nc.dram_tensor("enc", [128, 4 * H], f32, kind="ExternalInput")
    d_comb_w = nc.dram_tensor("comb_w", [128, 32 * 256], f32, kind="ExternalInput")
    d_comb_b = nc.dram_tensor("comb_b", [1, 256], f32, kind="ExternalInput")
    d_ih_w = nc.dram_tensor("ih_w", [128, 2 * 6144], f32, kind="ExternalInput")
    d_hh_w = nc.dram_tensor("hh_w", [128, 2 * 6144], f32, kind="ExternalInput")
    d_bias_rz = nc.dram_tensor("bias_rz", [128, 32], f32, kind="ExternalInput")
    d_bias_in = nc.dram_tensor("bias_in", [128, 16], f32, kind="ExternalInput")
    d_bias_hn = nc.dram_tensor("bias_hn", [128, 16], f32, kind="ExternalInput")
    d_out_w = nc.dram_tensor("out_w", [NT, 128, NKH * 512], f32, kind="ExternalInput")
    d_out_wt = nc.dram_tensor("out_wt", [128, NKH * TAIL], f32, kind="ExternalInput")
    d_out_b = nc.dram_tensor("out_b", [1, VS], f32, kind="ExternalInput")
    d_ones = nc.dram_tensor("ones", [1, 1], f32, kind="ExternalInput")

    d_logp = nc.dram_tensor("logp", [1, VS], f32, kind="ExternalOutput")
    d_h1 = nc.dram_tensor("h1", [1, H], f32, kind="ExternalOutput")
    d_attnw = nc.dram_tensor("attnw", [1, L], f32, kind="ExternalOutput")
    d_dbg = nc.dram_tensor("dbg", [N_CORES, 2], f32, kind="ExternalOutput")

    with tile.TileContext(nc) as tc:
        import contextlib
        ctx = contextlib.ExitStack()
        sb = ctx.enter_context(tc.tile_pool(name="sb", bufs=1))
        stg = ctx.enter_context(tc.tile_pool(name="stg", bufs=2))
        wch = ctx.enter_context(tc.tile_pool(name="wch", bufs=2))
        wout = ctx.enter_context(tc.tile_pool(name="wout", bufs=6))
        epool = ctx.enter_context(tc.tile_pool(name="epool", bufs=2))
        psA = ctx.enter_context(tc.tile_pool(name="psA", bufs=2, space="PSUM"))
        psB = ctx.enter_context(tc.tile_pool(name="psB", bufs=1, space="PSUM"))
        dram = ctx.enter_context(tc.tile_pool(name="dram", bufs=1, space="DRAM"))

        def dma(dst, src):
            return nc.sync.dma_start(dst, src)

        # ---- stage 0: small inputs ----
        emb_col = sb.tile([128, NKH], f32)
        dma(emb_col[:], d_emb_col[:, :])
        h0_col = sb.tile([128, NKH], f32)
        dma(h0_col[:], d_h0_col[:, :])
        h0sl_col = sb.tile([128, 2], f32)
        dma(h0sl_col[:], d_h0sl_col[:, :])
        attn_b = sb.tile([1, 512], f32)
        dma(attn_b[:], d_attn_b[:, :])
        comb_b = sb.tile([1, 256], f32)
        dma(comb_b[:], d_comb_b[:, :])
        bias_rz = sb.tile([128, 32], f32)
        dma(bias_rz[:], d_bias_rz[:, :])
        bias_in = sb.tile([128, 16], f32)
        dma(bias_in[:], d_bias_in[:, :])
        bias_hn = sb.tile([128, 16], f32)
        dma(bias_hn[:], d_bias_hn[:, :])
        out_b = sb.tile([1, VS], f32)
        dma(out_b[:], d_out_b[:, :])
        ones = sb.tile([1, 1], f32)
        dma(ones[:], d_ones[:, :])

        # ---- dummy collective to warm up the collective path ----
        dum_in = dram.tile([1, 2], f32)
        dum_out = dram.tile([N_CORES, 2], f32)
        dma(dum_in[:], attn_b[:, 0:2])
        nc.gpsimd.collective_compute(
            "AllGather", mybir.AluOpType.bypass, replica_groups=RG,
            ins=[dum_in[:].opt()], outs=[dum_out[:].opt()])
        dma(d_dbg[:, :], dum_out[:])

        # ---- stage 1: attn logits = [emb; h0] @ attn_W.T + attn_b ----
        def in_col(g):
            return emb_col[:, g - 16:g - 15] if g >= 16 else None

        ps_att = psA.tile([1, 512], f32, name="ps")
        nc.tensor.matmul(ps_att[:], lhsT=ones[:], rhs=attn_b[:],
                         start=True, stop=False)
        for t in range(4):           # 4 chain-weight tiles of [128, 4096]
            aw_t = wch.tile([128, 4096], wdt_ch, name="wchain")
            dma(aw_t[:], d_attn_w[:, t * 4096:(t + 1) * 4096].bitcast(wdt_ch))
            for j in range(8):       # 8 K-chunks per tile
                g = t * 8 + j        # global chunk over 2H
                lhs = emb_col[:, g:g + 1] if g < 16 else h0_col[:, g - 16:g - 15]
                if F32R_CHAIN:
                    lhs = lhs.bitcast(f32r)
                nc.tensor.matmul(ps_att[:], lhsT=lhs,
                                 rhs=aw_t[:, j * 512:(j + 1) * 512],
                                 start=False, stop=(g == 31))
        z_att = sb.tile([1, 512], f32)
        nc.scalar.copy(z_att[:], ps_att[:])

        # ---- stage 2: softmax over 512 ----
        nm_att = sb.tile([1, 1], f32)
        nc.vector.reduce_max(nm_att[:], z_att[:], axis=AX.X, negate=True)
        e_att = sb.tile([1, 512], f32)
        s_att = sb.tile([1, 1], f32)
        nc.scalar.activation(e_att[:], z_att[:], AF.Exp, bias=nm_att[:],
                             scale=1.0, accum_out=s_att[:])
        rinv = sb.tile([1, 1], f32)
        nc.vector.reciprocal(rinv[:], s_att[:])
        aw_row = sb.tile([1, 512], f32)
        nc.vector.tensor_scalar_mul(aw_row[:], e_att[:], rinv[:])
        dma(d_attnw[:, :], aw_row[:])
        # redistribute to column layout [128, 4]
        awb = dram.tile([1, 512], f32)
        dma(awb[:], aw_row[:])
        aw_col = sb.tile([128, 4], f32)
        dma(aw_col[:], awb[:].rearrange("a (c p) -> (a p) c", p=128))

        # ---- stage 3: attn_applied = aw @ enc  -> [1, 2048] ----
        ps_ap = psB.tile([1, H], f32, name="ps_ap")
        for t in range(2):           # 2 tiles of [128, 4096]
            enc_t = wch.tile([128, 4096], wdt_ch, name="wchain")
            dma(enc_t[:], d_enc[:, t * 4096:(t + 1) * 4096].bitcast(wdt_ch))
            for j in range(2):       # 2 L-chunks per tile
                c = t * 2 + j
                lhs = aw_col[:, c:c + 1]
                if F32R_CHAIN:
                    lhs = lhs.bitcast(f32r)
                for nsl in range(4):
                    nc.tensor.matmul(
                        ps_ap[:, nsl * 512:(nsl + 1) * 512],
                        lhsT=lhs,
                        rhs=enc_t[:, j * 2048 + nsl * 512:j * 2048 + (nsl + 1) * 512],
                        start=(c == 0), stop=(c == 3))
        ap_row = sb.tile([1, H], f32)
        nc.scalar.copy(ap_row[:], ps_ap[:])
        apb = dram.tile([1, H], f32)
        dma(apb[:], ap_row[:])
        ap_col = sb.tile([128, NKH], f32)
        dma(ap_col[:], apb[:].rearrange("a (c p) -> (a p) c", p=128))

        # ---- stage 4: x_k = relu([emb; ap] @ comb_W_k.T + comb_b_k) ----
        ps_x = psA.tile([1, 256], f32, name="ps")
        nc.tensor.matmul(ps_x[:], lhsT=ones[:], rhs=comb_b[:],
                         start=True, stop=False)
        for t in range(2):           # 2 tiles of [128, 4096] = 16 chunks each
            cw_t = wch.tile([128, 4096], wdt_ch, name="wchain")
            dma(cw_t[:], d_comb_w[:, t * 4096:(t + 1) * 4096].bitcast(wdt_ch))
            for j in range(16):
                g = t * 16 + j
                lhs = emb_col[:, g:g + 1] if g < 16 else ap_col[:, g - 16:g - 15]
                if F32R_CHAIN:
                    lhs = lhs.bitcast(f32r)
                nc.tensor.matmul(ps_x[:], lhsT=lhs,
                                 rhs=cw_t[:, j * 256:(j + 1) * 256],
                                 start=False, stop=(g == 31))
        x_row = sb.tile([1, 256], f32)
        nc.scalar.activation(x_row[:], ps_x[:], AF.Relu)
        xb = dram.tile([1, 256], f32)
        dma(xb[:], x_row[:])
        xsl_col = sb.tile([128, 2], f32)
        dma(xsl_col[:], xb[:].rearrange("a (c p) -> (a p) c", p=128))

        # ---- stage 5: GRU partial pre-gates (col-sharded) ----
        # host layout: [128, 12*1024], block n = [g0 512 | g1 512]
        arin = dram.tile([1, 2 * 6144], f32)
        ih_t = hh_t = None
        for n in range(12):
            if n % 4 == 0:
                ih_t = wch.tile([128, 4096], wdt_ch, name="wchain")
                dma(ih_t[:], d_ih_w[:, n * 1024:(n + 4) * 1024].bitcast(wdt_ch))
                hh_t = wch.tile([128, 4096], wdt_ch, name="wchain")
                dma(hh_t[:], d_hh_w[:, n * 1024:(n + 4) * 1024].bitcast(wdt_ch))
            off = (n % 4) * 1024
            ps_gi = psA.tile([1, 512], f32, name="ps")
            for g in range(2):
                lhs = xsl_col[:, g:g + 1]
                if F32R_CHAIN:
                    lhs = lhs.bitcast(f32r)
                nc.tensor.matmul(ps_gi[:], lhsT=lhs,
                                 rhs=ih_t[:, off + g * 512:off + (g + 1) * 512],
                                 start=(g == 0), stop=(g == 1))
            st_i = stg.tile([1, 512], f32, name="stg")
            nc.scalar.copy(st_i[:], ps_gi[:])
            dma(arin[:, n * 512:(n + 1) * 512], st_i[:])
            ps_gh = psA.tile([1, 512], f32, name="ps")
            for g in range(2):
                lhs = h0sl_col[:, g:g + 1]
                if F32R_CHAIN:
                    lhs = lhs.bitcast(f32r)
                nc.tensor.matmul(ps_gh[:], lhsT=lhs,
                                 rhs=hh_t[:, off + g * 512:off + (g + 1) * 512],
                                 start=(g == 0), stop=(g == 1))
            st_h = stg.tile([1, 512], f32, name="stg")
            nc.vector.tensor_copy(st_h[:], ps_gh[:])
            dma(arin[:, 6144 + n * 512:6144 + (n + 1) * 512], st_h[:])

        # ---- stage 6: AllReduce partial pre-gates ----
        arout = dram.tile([1, 2 * 6144], f32)
        nc.gpsimd.collective_compute(
            "AllReduce", ALU.add, replica_groups=RG,
            ins=[arin[:].opt()], outs=[arout[:].opt()])
        gi_col = sb.tile([128, 48], f32)
        dma(gi_col[:], arout[:, 0:6144].rearrange("a (c p) -> (a p) c", p=128))
        gh_col = sb.tile([128, 48], f32)
        dma(gh_col[:], arout[:, 6144:12288].rearrange("a (c p) -> (a p) c", p=128))

        # ---- stage 7: gates + h1 (column layout [128, 16]) ----
        t_rz = sb.tile([128, 32], f32)
        nc.vector.tensor_add(t_rz[:], gi_col[:, 0:32], gh_col[:, 0:32])
        nc.vector.tensor_add(t_rz[:], t_rz[:], bias_rz[:])
        rz = sb.tile([128, 32], f32)
        nc.scalar.activation(rz[:], t_rz[:], AF.Sigmoid)
        hn = sb.tile([128, 16], f32)
        nc.vector.tensor_add(hn[:], gh_col[:, 32:48], bias_hn[:])
        rhn = sb.tile([128, 16], f32)
        nc.vector.tensor_mul(rhn[:], rz[:, 0:16], hn[:])
        tn = sb.tile([128, 16], f32)
        nc.vector.tensor_add(tn[:], gi_col[:, 32:48], bias_in[:])
        nc.vector.tensor_add(tn[:], tn[:], rhn[:])
        n_g = sb.tile([128, 16], f32)
        nc.scalar.activation(n_g[:], tn[:], AF.Tanh)
        dmn = sb.tile([128, 16], f32)
        nc.vector.tensor_tensor(out=dmn[:], in0=h0_col[:], in1=n_g[:],
                                op=ALU.subtract)
        zd = sb.tile([128, 16], f32)
        nc.vector.tensor_mul(zd[:], rz[:, 16:32], dmn[:])
        h1_col = sb.tile([128, NKH], f32)
        nc.vector.tensor_add(h1_col[:], n_g[:], zd[:])
        dma(d_h1[0:1, :].rearrange("a (c p) -> (a p) c", p=128), h1_col[:])
        if F32R_OUT:
            h1_lhs = sb.tile([128, NKH], f32r)
            nc.vector.tensor_copy(h1_lhs[:], h1_col[:])
        else:
            h1_lhs = h1_col

        # ---- stage 8: output projection, streamed ----
        z_row = sb.tile([1, NT * 512 + TAIL], f32)
        nm_buf = sb.tile([1, 16], f32)
        s_buf = sb.tile([1, 16], f32)

        def z_tile_stats(n, zt):
            nmt = nm_buf[:, n:n + 1]
            nc.vector.reduce_max(nmt, zt, axis=AX.X, negate=True)
            etile = epool.tile([1, 512], f32, name="etile")
            nc.scalar.activation(etile[:, 0:zt.shape[-1]], zt, AF.Exp,
                                 bias=nmt, scale=1.0,
                                 accum_out=s_buf[:, n:n + 1])

        for n in range(NT):
            for half in range(2):
                w_t = wout.tile([128, 4096], wdt_out, name="wout")
                dma(w_t[:], d_out_w[n][:, half * 4096:(half + 1) * 4096]
                    .bitcast(wdt_out))
                if half == 0:
                    ps_z = psA.tile([1, 512], f32, name="ps")
                    nc.tensor.matmul(ps_z[:], lhsT=ones[:],
                                     rhs=out_b[:, n * 512:(n + 1) * 512],
                                     start=True, stop=False)
                for j in range(8):
                    g = half * 8 + j
                    nc.tensor.matmul(ps_z[:], lhsT=h1_lhs[:, g:g + 1],
                                     rhs=w_t[:, j * 512:(j + 1) * 512],
                                     start=False, stop=(g == 15))
            zt = z_row[:, n * 512:(n + 1) * 512]
            nc.scalar.copy(zt, ps_z[:])
            z_tile_stats(n, zt)

        # tail tile (N=139, always fp32)
        wtl = wout.tile([128, NKH * TAIL], f32, name="wout")
        dma(wtl[:], d_out_wt[:, :])
        ps_zt = psA.tile([1, TAIL], f32, name="ps")
        nc.tensor.matmul(ps_zt[:], lhsT=ones[:], rhs=out_b[:, NT * 512:VS],
                         start=True, stop=False)
        for g in range(NKH):
            nc.tensor.matmul(ps_zt[:], lhsT=h1_col[:, g:g + 1],
                             rhs=wtl[:, g * TAIL:(g + 1) * TAIL],
                             start=False, stop=(g == NKH - 1))
        ztl = z_row[:, NT * 512:NT * 512 + TAIL]
        nc.scalar.copy(ztl, ps_zt[:])
        z_tile_stats(NT, ztl)

        # ---- stage 9: local logsumexp combine ----
        nmk = sb.tile([1, 1], f32)
        nc.vector.tensor_reduce(nmk[:], nm_buf[:, 0:NT + 1], axis=AX.X,
                                op=ALU.min)
        wgt = sb.tile([1, NT + 1], f32)
        nc.scalar.activation(wgt[:], nm_buf[:, 0:NT + 1], AF.Exp,
                             bias=nmk[:], scale=-1.0)
        sw = sb.tile([1, NT + 1], f32)
        nc.vector.tensor_mul(sw[:], wgt[:], s_buf[:, 0:NT + 1])
        sk = sb.tile([1, 1], f32)
        nc.vector.reduce_sum(sk[:], sw[:], axis=AX.X)
        msk = sb.tile([1, 2], f32)
        nc.vector.tensor_copy(msk[:, 0:1], nmk[:])
        nc.vector.tensor_copy(msk[:, 1:2], sk[:])

        # ---- stage 10: AllGather (m, s) + global normalizer ----
        agin = dram.tile([1, 2], f32)
        dma(agin[:], msk[:])
        agout = dram.tile([N_CORES, 2], f32)
        nc.gpsimd.collective_compute(
            "AllGather", mybir.AluOpType.bypass, replica_groups=RG,
            ins=[agin[:].opt()], outs=[agout[:].opt()])
        ms_all = sb.tile([1, 2 * N_CORES], f32)
        dma(ms_all[:], agout[:].rearrange("(a r) t -> a (r t)", a=1))
        ms_v = ms_all[:].rearrange("a (r t) -> a t r", t=2)
        nm_all = ms_v[:, 0, :]
        s_all = ms_v[:, 1, :]
        nmg = sb.tile([1, 1], f32)
        nc.vector.tensor_reduce(nmg[:], nm_all, axis=AX.X, op=ALU.min)
        wr = sb.tile([1, N_CORES], f32)
        nc.scalar.activation(wr[:], nm_all, AF.Exp, bias=nmg[:], scale=-1.0)
        swr = sb.tile([1, N_CORES], f32)
        nc.vector.tensor_mul(swr[:], wr[:], s_all)
        sg = sb.tile([1, 1], f32)
        nc.vector.reduce_sum(sg[:], swr[:], axis=AX.X)
        logs = sb.tile([1, 1], f32)
        nc.scalar.activation(logs[:], sg[:], AF.Ln)
        nlogz = sb.tile([1, 1], f32)
        nc.vector.tensor_tensor(out=nlogz[:], in0=nmg[:], in1=logs[:],
                                op=ALU.subtract)   # -logZ = nmg - logS

        # ---- stage 11: final log-probs (in place on z_row) ----
        nc.scalar.activation(z_row[:, 0:VS], z_row[:, 0:VS], AF.Identity,
                             bias=nlogz[:], scale=1.0)
        dma(d_logp[:, :], z_row[:, 0:VS])

        ctx.close()

    nc.compile()
    return nc


def _prep_inputs(inputs):
    inp = {k: np.asarray(v) for k, v in inputs.items()}
    idx = int(np.asarray(inp["input"]).reshape(-1)[0])
    emb_row = inp["emb"][idx].astype(np.float32)          # [H]
    h0 = inp["hidden"].reshape(H).astype(np.float32)

    def col(v):
        return np.ascontiguousarray(v.reshape(-1, 128).T)

    emb_col = col(emb_row)                                # [128, NKH]
    h0_col = col(h0)

    attn_W = inp["attn_W"].astype(np.float32)             # [512, 4096]
    attn_w = np.ascontiguousarray(
        attn_W.reshape(512, 32, 128).transpose(2, 1, 0)).reshape(128, 32 * 512)
    enc = inp["encoder_outputs"].astype(np.float32)       # [512, 2048]
    enc_f = np.ascontiguousarray(
        enc.reshape(4, 128, H).transpose(1, 0, 2)).reshape(128, 4 * H)

    comb_W = inp["comb_W"].astype(np.float32)             # [2048, 4096]
    W_ih = inp["W_ih"].astype(np.float32)                 # [6144, 2048]
    W_hh = inp["W_hh"].astype(np.float32)
    b_ih = inp["b_ih"].astype(np.float32)
    b_hh = inp["b_hh"].astype(np.float32)
    out_W = inp["out_W"].astype(np.float32)               # [V, 2048]
    out_b = inp["out_b"].astype(np.float32)

    bias_rz = col((b_ih[:4096] + b_hh[:4096]))            # [128, 32]
    bias_in = col(b_ih[4096:6144])                        # [128, 16]
    bias_hn = col(b_hh[4096:6144])

    base = {
        "emb_col": emb_col, "h0_col": h0_col,
        "attn_w": attn_w, "attn_b": inp["attn_b"].astype(np.float32)[None, :],
        "enc": enc_f,
        "bias_rz": bias_rz, "bias_in": bias_in, "bias_hn": bias_hn,
        "ones": np.ones((1, 1), np.float32),
    }

    # padded out_W / out_b
    P = N_CORES * VS
    out_W_pad = np.zeros((P, H), np.float32)
    out_W_pad[:V] = out_W
    out_b_pad = np.full((P,), NEG_BIG, np.float32)
    out_b_pad[:V] = out_b

    in_maps = []
    for k in range(N_CORES):
        m = dict(base)
        sl = slice(256 * k, 256 * (k + 1))
        m["h0sl_col"] = np.ascontiguousarray(h0[sl].reshape(2, 128).T)
        Rk = comb_W[sl]                                   # [256, 4096]
        m["comb_w"] = np.ascontiguousarray(
            Rk.reshape(256, 32, 128).transpose(2, 1, 0)).reshape(128, 32 * 256)
        m["comb_b"] = inp["comb_b"].astype(np.float32)[None, sl]
        # interleaved: [128, 12*1024], block n = [g0 512 | g1 512]
        Ck = W_ih[:, sl]                                  # [6144, 256]
        m["ih_w"] = np.ascontiguousarray(
            Ck.reshape(12, 512, 2, 128).transpose(3, 0, 2, 1)).reshape(128, 2 * 6144)
        Dk = W_hh[:, sl]
        m["hh_w"] = np.ascontiguousarray(
            Dk.reshape(12, 512, 2, 128).transpose(3, 0, 2, 1)).reshape(128, 2 * 6144)
        v0 = k * VS
        Mk = out_W_pad[v0:v0 + NT * 512]                  # [6144, 2048]
        m["out_w"] = np.ascontiguousarray(
            Mk.reshape(NT, 512, NKH, 128).transpose(0, 3, 2, 1)
        ).reshape(NT, 128, NKH * 512)
        Tk = out_W_pad[v0 + NT * 512:v0 + VS]             # [139, 2048]
        m["out_wt"] = np.ascontiguousarray(
            Tk.reshape(TAIL, NKH, 128).transpose(2, 1, 0)).reshape(128, NKH * TAIL)
        m["out_b"] = out_b_pad[None, v0:v0 + VS]
        in_maps.append(m)
    return in_maps


def kernel(**inputs):
    sys.path.insert(0, "/opt/trn_rl_repo")
    from concourse.bass_utils import run_bass_kernel_spmd

    if "nc" not in _CACHE:
        _CACHE["nc"] = _build()
    nc = _CACHE["nc"]

    in_maps = _prep_inputs(inputs)
    res = run_bass_kernel_spmd(nc, in_maps, core_ids=list(range(N_CORES)))
    r = res.results

    logp = np.concatenate([r[k]["logp"] for k in range(N_CORES)],
                          axis=1)[:, :V].astype(np.float32)
    h1 = r[0]["h1"].reshape(1, 1, H).astype(np.float32)
    attnw = r[0]["attnw"].astype(np.float32)
    return (logp, h1, attnw)


if __name__ == "__main__":
    sys.path.insert(0, os.path.dirname(os.path.abspath(__file__)))
    import reference
    inputs = {k: np.asarray(v) for k, v in reference.setup_inputs().items()}
    out = kernel(**inputs)
    ref = reference.reference(**inputs)
    for name, a, b in zip(("logp", "h1", "attnw"), out, ref):
        a, b = np.asarray(a), np.asarray(b)
        scale = max(np.abs(b).max(), 1e-30)
        print(f"{name}: max abs err {np.abs(a - b).max():.3e} "
              f"(rel {np.abs(a - b).max() / scale:.3e})")


# revision 13
# speedup vs baseline: 1.0899x; 1.0899x over previous
"""AttnDecoderRNN single-step on 8 axon-tunneled TRN2 NeuronCores.

Strategy (tensor parallel over 8 cores, batch=1):
  - embedding: host gathers the single row emb[input] (only 8KB read).
  - attention (attn_W [512,4096], encoder_outputs [512,2048]): replicated on
    every core (small weights; avoids two collectives).
  - comb_W [2048,4096]: row-sharded (core k computes x_k = x[256k:256k+256]).
  - GRU W_ih/W_hh [6144,2048]: column-sharded with slices matched to x_k /
    h0_k, producing partial pre-gates; one AllReduce(48KB) sums them, then
    every core computes the full h1 redundantly (cheap elementwise).
  - out_W [50257,2048]: vocab-sharded (6283 rows/core, padded), streamed from
    HBM in a flat per-partition-contiguous layout; logits z_k per core with
    per-tile max / sum-exp stats; tail AllGather(16B) of (max, sumexp) gives
    the global log-normalizer; each core emits its log_prob shard.
  - host concatenates shards and trims padding.

All activation vectors live in "column layout" [128, C] (element i at
partition i%128, column i//128) so they feed matmul lhsT chunks [128,1]
directly. PE matmuls run fp32 (exact 2-pass mode) for the serial chain and
float32r (1 cyc/col) for the big streamed output projection by default.
"""

import os
import sys

import numpy as np

H = 2048
L = 512
V = 50257
N_CORES = 8
VS = 6283            # vocab rows per core (8*6283 = 50264 >= 50257)
NT = 12              # full 512-wide output tiles per core
TAIL = VS - NT * 512  # 139
NKH = H // 128       # 16 chunks over H
NEG_BIG = -1.0e30

F32R_OUT = os.environ.get("BASS_F32R_OUT", "1") == "1"
F32R_CHAIN = os.environ.get("BASS_F32R_CHAIN", "0") == "1"

_CACHE = {}


def _build():
    sys.path.insert(0, "/opt/trn_rl_repo")
    import concourse.bass as bass  # noqa: F401
    import concourse.tile as tile
    from concourse import bacc, mybir

    f32 = mybir.dt.float32
    f32r = mybir.dt.float32r
    AF = mybir.ActivationFunctionType
    AX = mybir.AxisListType
    ALU = mybir.AluOpType
    RG = [list(range(N_CORES))]

    wdt_out = f32r if F32R_OUT else f32
    wdt_ch = f32r if F32R_CHAIN else f32

    nc = bacc.Bacc("TRN2", target_bir_lowering=False, debug=False,
                   num_devices=N_CORES)

    # ---- I/O ----
    d_emb_col = nc.dram_tensor("emb_col", [128, NKH], f32, kind="ExternalInput")
    d_h0_col = nc.dram_tensor("h0_col", [128, NKH], f32, kind="ExternalInput")
    d_h0sl_col = nc.dram_tensor("h0sl_col", [128, 2], f32, kind="ExternalInput")
    d_attn_w = nc.dram_tensor("attn_w", [128, 32 * 512], f32, kind="ExternalInput")
    d_attn_b = nc.dram_tensor("attn_b", [1, 512], f32, kind="ExternalInput")
    d_enc = nc.dram_tensor("enc", [128, 4 * H], f32, kind="ExternalInput")
    d_comb_w = nc.dram_tensor("comb_w", [128, 32 * 256], f32, kind="ExternalInput")
    d_comb_b = nc.dram_tensor("comb_b", [128, 2], f32, kind="ExternalInput")
    d_ih_w = nc.dram_tensor("ih_w", [128, 2 * 6144], f32, kind="ExternalInput")
    d_hh_w = nc.dram_tensor("hh_w", [128, 2 * 6144], f32, kind="ExternalInput")
    d_bias_rz = nc.dram_tensor("bias_rz", [128, 32], f32, kind="ExternalInput")
    d_bias_in = nc.dram_tensor("bias_in", [128, 16], f32, kind="ExternalInput")
    d_bias_hn = nc.dram_tensor("bias_hn", [128, 16], f32, kind="ExternalInput")
    d_out_w = nc.dram_tensor("out_w", [NT, 128, NKH * 512], f32, kind="ExternalInput")
    d_out_wt = nc.dram_tensor("out_wt", [128, NKH * TAIL], f32, kind="ExternalInput")
    d_out_b = nc.dram_tensor("out_b", [1, VS], f32, kind="ExternalInput")
    d_ones = nc.dram_tensor("ones", [1, 1], f32, kind="ExternalInput")

    d_logp = nc.dram_tensor("logp", [1, VS], f32, kind="ExternalOutput")
    d_h1 = nc.dram_tensor("h1", [1, H], f32, kind="ExternalOutput")
    d_attnw = nc.dram_tensor("attnw", [1, L], f32, kind="ExternalOutput")
    d_dbg = nc.dram_tensor("dbg", [N_CORES, 2], f32, kind="ExternalOutput")

    with tile.TileContext(nc) as tc:
        import contextlib
        ctx = contextlib.ExitStack()
        sb = ctx.enter_context(tc.tile_pool(name="sb", bufs=1))
        wch = ctx.enter_context(tc.tile_pool(name="wch", bufs=2))
        wout = ctx.enter_context(tc.tile_pool(name="wout", bufs=6))
        epool = ctx.enter_context(tc.tile_pool(name="epool", bufs=2))
        psA = ctx.enter_context(tc.tile_pool(name="psA", bufs=2, space="PSUM"))
        psB = ctx.enter_context(tc.tile_pool(name="psB", bufs=1, space="PSUM"))
        dram = ctx.enter_context(tc.tile_pool(name="dram", bufs=1, space="DRAM"))

        def dma(dst, src):
            return nc.sync.dma_start(dst, src)

        # ---- stage 0: small inputs ----
        emb_col = sb.tile([128, NKH], f32)
        dma(emb_col[:], d_emb_col[:, :])
        h0_col = sb.tile([128, NKH], f32)
        dma(h0_col[:], d_h0_col[:, :])
        h0sl_col = sb.tile([128, 2], f32)
        dma(h0sl_col[:], d_h0sl_col[:, :])
        attn_b = sb.tile([1, 512], f32)
        dma(attn_b[:], d_attn_b[:, :])
        comb_b_col = sb.tile([128, 2], f32)
        dma(comb_b_col[:], d_comb_b[:, :])
        bias_rz = sb.tile([128, 32], f32)
        dma(bias_rz[:], d_bias_rz[:, :])
        bias_in = sb.tile([128, 16], f32)
        dma(bias_in[:], d_bias_in[:, :])
        bias_hn = sb.tile([128, 16], f32)
        dma(bias_hn[:], d_bias_hn[:, :])
        out_b = sb.tile([1, VS], f32)
        dma(out_b[:], d_out_b[:, :])
        ones = sb.tile([1, 1], f32)
        dma(ones[:], d_ones[:, :])

        # ---- dummy collective to warm up the collective path ----
        dum_in = dram.tile([1, 2], f32)
        dum_out = dram.tile([N_CORES, 2], f32)
        dma(dum_in[:], attn_b[:, 0:2])
        nc.gpsimd.collective_compute(
            "AllGather", mybir.AluOpType.bypass, replica_groups=RG,
            ins=[dum_in[:].opt()], outs=[dum_out[:].opt()])
        nc.gpsimd.dma_start(d_dbg[:, :], dum_out[:])

        # ---- stage 1: attn logits = [emb; h0] @ attn_W.T + attn_b ----
        ps_att = psA.tile([1, 512], f32, name="ps")
        nc.tensor.matmul(ps_att[:], lhsT=ones[:], rhs=attn_b[:],
                         start=True, stop=False)
        for t in range(4):           # 4 chain-weight tiles of [128, 4096]
            aw_t = wch.tile([128, 4096], wdt_ch, name="wchain")
            dma(aw_t[:], d_attn_w[:, t * 4096:(t + 1) * 4096].bitcast(wdt_ch))
            for j in range(8):       # 8 K-chunks per tile
                g = t * 8 + j        # global chunk over 2H
                lhs = emb_col[:, g:g + 1] if g < 16 else h0_col[:, g - 16:g - 15]
                if F32R_CHAIN:
                    lhs = lhs.bitcast(f32r)
                nc.tensor.matmul(ps_att[:], lhsT=lhs,
                                 rhs=aw_t[:, j * 512:(j + 1) * 512],
                                 start=False, stop=(g == 31))
        z_att = sb.tile([1, 512], f32)
        nc.scalar.copy(z_att[:], ps_att[:])

        # ---- stage 2: softmax over 512 ----
        nm_att = sb.tile([1, 1], f32)
        nc.vector.reduce_max(nm_att[:], z_att[:], axis=AX.X, negate=True)
        e_att = sb.tile([1, 512], f32)
        s_att = sb.tile([1, 1], f32)
        nc.scalar.activation(e_att[:], z_att[:], AF.Exp, bias=nm_att[:],
                             scale=1.0, accum_out=s_att[:])
        rinv = sb.tile([1, 1], f32)
        nc.vector.reciprocal(rinv[:], s_att[:])
        aw_row = sb.tile([1, 512], f32)
        nc.vector.tensor_scalar_mul(aw_row[:], e_att[:], rinv[:])
        nc.gpsimd.dma_start(d_attnw[:, :], aw_row[:])
        # transpose to column layout [128, 4] via PE (identity = ones [1,1])
        ps_awT = psA.tile([128, 4], f32, name="ps")
        for c in range(4):
            nc.tensor.matmul(ps_awT[:, c:c + 1], lhsT=aw_row[:, c * 128:(c + 1) * 128],
                             rhs=ones[:], is_transpose=True, start=True, stop=True)
        aw_col = sb.tile([128, 4], f32)
        nc.vector.tensor_copy(aw_col[:], ps_awT[:])

        # ---- stage 3: attn_applied in column layout [128, 16] ----
        # enc host layout: [128, (c*4+j)*128+q] = enc[128j+q, 128c+p]
        ps_apc = psB.tile([128, NKH], f32, name="ps_ap")
        for t in range(2):           # 2 tiles of [128, 4096], cols c in [8t, 8t+8)
            enc_t = wch.tile([128, 4096], wdt_ch, name="wchain")
            dma(enc_t[:], d_enc[:, t * 4096:(t + 1) * 4096].bitcast(wdt_ch))
            for cc in range(8):
                c = t * 8 + cc
                for j in range(4):
                    off = (cc * 4 + j) * 128
                    nc.tensor.matmul(ps_apc[:, c:c + 1],
                                     lhsT=enc_t[:, off:off + 128],
                                     rhs=aw_col[:, j:j + 1],
                                     start=(j == 0), stop=(j == 3))
        ap_col = sb.tile([128, NKH], f32)
        nc.vector.tensor_copy(ap_col[:], ps_apc[:])

        # ---- stage 4: x_k in column layout [128, 2] ----
        # comb host layout: [kk, (c*32+g)*128+q] = Rk[128c+q, 128g+kk]
        ps_xc = psA.tile([128, 2], f32, name="ps")
        for t in range(2):           # tile t = col c = t (32 chunks each)
            cw_t = wch.tile([128, 4096], wdt_ch, name="wchain")
            dma(cw_t[:], d_comb_w[:, t * 4096:(t + 1) * 4096].bitcast(wdt_ch))
            for g in range(32):
                rhs_v = emb_col[:, g:g + 1] if g < 16 else ap_col[:, g - 16:g - 15]
                nc.tensor.matmul(ps_xc[:, t:t + 1],
                                 lhsT=cw_t[:, g * 128:(g + 1) * 128],
                                 rhs=rhs_v,
                                 start=(g == 0), stop=(g == 31))
        xbias = sb.tile([128, 2], f32)
        nc.vector.tensor_add(xbias[:], ps_xc[:], comb_b_col[:])
        xsl_col = sb.tile([128, 2], f32)
        nc.scalar.activation(xsl_col[:], xbias[:], AF.Relu)

        # ---- stage 5: GRU partial pre-gates, column layout [128, 48] ----
        # host layout: [kk, (c*2+g)*128+q] = Ck[128c+q, 128g+kk]
        arin = dram.tile([128, 96], f32)
        ps_gic = psB.tile([128, 48], f32, name="ps_gi")
        ps_ghc = psB.tile([128, 48], f32, name="ps_gh")
        for t in range(3):           # ih tile t = cols c in [16t, 16t+16)
            ih_t = wch.tile([128, 4096], wdt_ch, name="wchain")
            dma(ih_t[:], d_ih_w[:, t * 4096:(t + 1) * 4096].bitcast(wdt_ch))
            for cc in range(16):
                c = t * 16 + cc
                for g in range(2):
                    off = (cc * 2 + g) * 128
                    nc.tensor.matmul(ps_gic[:, c:c + 1],
                                     lhsT=ih_t[:, off:off + 128],
                                     rhs=xsl_col[:, g:g + 1],
                                     start=(g == 0), stop=(g == 1))
        pg_col = sb.tile([128, 96], f32)
        nc.vector.tensor_copy(pg_col[:, 0:48], ps_gic[:])
        for t in range(3):
            hh_t = wch.tile([128, 4096], wdt_ch, name="wchain")
            dma(hh_t[:], d_hh_w[:, t * 4096:(t + 1) * 4096].bitcast(wdt_ch))
            for cc in range(16):
                c = t * 16 + cc
                for g in range(2):
                    off = (cc * 2 + g) * 128
                    nc.tensor.matmul(ps_ghc[:, c:c + 1],
                                     lhsT=hh_t[:, off:off + 128],
                                     rhs=h0sl_col[:, g:g + 1],
                                     start=(g == 0), stop=(g == 1))
        nc.scalar.copy(pg_col[:, 48:96], ps_ghc[:])
        nc.gpsimd.dma_start(arin[:], pg_col[:])

        # ---- stage 6: AllReduce partial pre-gates ----
        arout = dram.tile([128, 96], f32)
        nc.gpsimd.collective_compute(
            "AllReduce", ALU.add, replica_groups=RG,
            ins=[arin[:].opt()], outs=[arout[:].opt()])
        garr = sb.tile([128, 96], f32)
        nc.gpsimd.dma_start(garr[:], arout[:])
        gi_col = garr[:, 0:48]
        gh_col = garr[:, 48:96]

        # ---- stage 7: gates + h1 (column layout [128, 16]) ----
        t_rz = sb.tile([128, 32], f32)
        nc.vector.tensor_add(t_rz[:], gi_col[:, 0:32], gh_col[:, 0:32])
        nc.vector.tensor_add(t_rz[:], t_rz[:], bias_rz[:])
        rz = sb.tile([128, 32], f32)
        nc.scalar.activation(rz[:], t_rz[:], AF.Sigmoid)
        hn = sb.tile([128, 16], f32)
        nc.vector.tensor_add(hn[:], gh_col[:, 32:48], bias_hn[:])
        rhn = sb.tile([128, 16], f32)
        nc.vector.tensor_mul(rhn[:], rz[:, 0:16], hn[:])
        tn = sb.tile([128, 16], f32)
        nc.vector.tensor_add(tn[:], gi_col[:, 32:48], bias_in[:])
        nc.vector.tensor_add(tn[:], tn[:], rhn[:])
        n_g = sb.tile([128, 16], f32)
        nc.scalar.activation(n_g[:], tn[:], AF.Tanh)
        dmn = sb.tile([128, 16], f32)
        nc.vector.tensor_tensor(out=dmn[:], in0=h0_col[:], in1=n_g[:],
                                op=ALU.subtract)
        zd = sb.tile([128, 16], f32)
        nc.vector.tensor_mul(zd[:], rz[:, 16:32], dmn[:])
        h1_col = sb.tile([128, NKH], f32)
        nc.vector.tensor_add(h1_col[:], n_g[:], zd[:])
        nc.gpsimd.dma_start(d_h1[0:1, :].rearrange("a (c p) -> (a p) c", p=128), h1_col[:])
        if F32R_OUT:
            h1_lhs = sb.tile([128, NKH], f32r)
            nc.vector.tensor_copy(h1_lhs[:], h1_col[:])
        else:
            h1_lhs = h1_col

        # ---- stage 8: output projection, streamed ----
        z_row = sb.tile([1, NT * 512 + TAIL], f32)
        nm_buf = sb.tile([1, 16], f32)
        s_buf = sb.tile([1, 16], f32)

        def z_tile_stats(n, zt):
            nmt = nm_buf[:, n:n + 1]
            nc.vector.reduce_max(nmt, zt, axis=AX.X, negate=True)
            etile = epool.tile([1, 512], f32, name="etile")
            nc.scalar.activation(etile[:, 0:zt.shape[-1]], zt, AF.Exp,
                                 bias=nmt, scale=1.0,
                                 accum_out=s_buf[:, n:n + 1])

        for n in range(NT):
            for half in range(2):
                w_t = wout.tile([128, 4096], wdt_out, name="wout")
                dma(w_t[:], d_out_w[n][:, half * 4096:(half + 1) * 4096]
                    .bitcast(wdt_out))
                if half == 0:
                    ps_z = psA.tile([1, 512], f32, name="ps")
                    nc.tensor.matmul(ps_z[:], lhsT=ones[:],
                                     rhs=out_b[:, n * 512:(n + 1) * 512],
                                     start=True, stop=False)
                for j in range(8):
                    g = half * 8 + j
                    nc.tensor.matmul(ps_z[:], lhsT=h1_lhs[:, g:g + 1],
                                     rhs=w_t[:, j * 512:(j + 1) * 512],
                                     start=False, stop=(g == 15))
            zt = z_row[:, n * 512:(n + 1) * 512]
            nc.scalar.copy(zt, ps_z[:])
            z_tile_stats(n, zt)

        # tail tile (N=139, always fp32)
        wtl = wout.tile([128, NKH * TAIL], f32, name="wout")
        dma(wtl[:], d_out_wt[:, :])
        ps_zt = psA.tile([1, TAIL], f32, name="ps")
        nc.tensor.matmul(ps_zt[:], lhsT=ones[:], rhs=out_b[:, NT * 512:VS],
                         start=True, stop=False)
        for g in range(NKH):
            nc.tensor.matmul(ps_zt[:], lhsT=h1_col[:, g:g + 1],
                             rhs=wtl[:, g * TAIL:(g + 1) * TAIL],
                             start=False, stop=(g == NKH - 1))
        ztl = z_row[:, NT * 512:NT * 512 + TAIL]
        nc.scalar.copy(ztl, ps_zt[:])
        z_tile_stats(NT, ztl)

        # ---- stage 9: local logsumexp combine ----
        nmk = sb.tile([1, 1], f32)
        nc.vector.tensor_reduce(nmk[:], nm_buf[:, 0:NT + 1], axis=AX.X,
                                op=ALU.min)
        wgt = sb.tile([1, NT + 1], f32)
        nc.scalar.activation(wgt[:], nm_buf[:, 0:NT + 1], AF.Exp,
                             bias=nmk[:], scale=-1.0)
        sw = sb.tile([1, NT + 1], f32)
        nc.vector.tensor_mul(sw[:], wgt[:], s_buf[:, 0:NT + 1])
        sk = sb.tile([1, 1], f32)
        nc.vector.reduce_sum(sk[:], sw[:], axis=AX.X)
        msk = sb.tile([1, 2], f32)
        nc.vector.tensor_copy(msk[:, 0:1], nmk[:])
        nc.vector.tensor_copy(msk[:, 1:2], sk[:])

        # ---- stage 10: AllGather (m, s) + global normalizer ----
        agin = dram.tile([1, 2], f32)
        dma(agin[:], msk[:])
        agout = dram.tile([N_CORES, 2], f32)
        nc.gpsimd.collective_compute(
            "AllGather", mybir.AluOpType.bypass, replica_groups=RG,
            ins=[agin[:].opt()], outs=[agout[:].opt()])
        ms_all = sb.tile([1, 2 * N_CORES], f32)
        dma(ms_all[:], agout[:].rearrange("(a r) t -> a (r t)", a=1))
        ms_v = ms_all[:].rearrange("a (r t) -> a t r", t=2)
        nm_all = ms_v[:, 0, :]
        s_all = ms_v[:, 1, :]
        nmg = sb.tile([1, 1], f32)
        nc.vector.tensor_reduce(nmg[:], nm_all, axis=AX.X, op=ALU.min)
        wr = sb.tile([1, N_CORES], f32)
        nc.scalar.activation(wr[:], nm_all, AF.Exp, bias=nmg[:], scale=-1.0)
        swr = sb.tile([1, N_CORES], f32)
        nc.vector.tensor_mul(swr[:], wr[:], s_all)
        sg = sb.tile([1, 1], f32)
        nc.vector.reduce_sum(sg[:], swr[:], axis=AX.X)
        logs = sb.tile([1, 1], f32)
        nc.scalar.activation(logs[:], sg[:], AF.Ln)
        nlogz = sb.tile([1, 1], f32)
        nc.vector.tensor_tensor(out=nlogz[:], in0=nmg[:], in1=logs[:],
                                op=ALU.subtract)   # -logZ = nmg - logS

        # ---- stage 11: final log-probs (in place on z_row) ----
        nc.scalar.activation(z_row[:, 0:VS], z_row[:, 0:VS], AF.Identity,
                             bias=nlogz[:], scale=1.0)
        dma(d_logp[:, :], z_row[:, 0:VS])

        ctx.close()

    nc.compile()
    return nc


def _prep_inputs(inputs):
    inp = {k: np.asarray(v) for k, v in inputs.items()}
    idx = int(np.asarray(inp["input"]).reshape(-1)[0])
    emb_row = inp["emb"][idx].astype(np.float32)          # [H]
    h0 = inp["hidden"].reshape(H).astype(np.float32)

    def col(v):
        return np.ascontiguousarray(v.reshape(-1, 128).T)

    emb_col = col(emb_row)                                # [128, NKH]
    h0_col = col(h0)

    attn_W = inp["attn_W"].astype(np.float32)             # [512, 4096]
    attn_w = np.ascontiguousarray(
        attn_W.reshape(512, 32, 128).transpose(2, 1, 0)).reshape(128, 32 * 512)
    enc = inp["encoder_outputs"].astype(np.float32)       # [512, 2048]
    # [kk, (c*4+j)*128+p] = enc[128j+kk, 128c+p]
    enc_f = np.ascontiguousarray(
        enc.reshape(4, 128, NKH, 128).transpose(1, 2, 0, 3)).reshape(128, 4 * H)

    comb_W = inp["comb_W"].astype(np.float32)             # [2048, 4096]
    W_ih = inp["W_ih"].astype(np.float32)                 # [6144, 2048]
    W_hh = inp["W_hh"].astype(np.float32)
    b_ih = inp["b_ih"].astype(np.float32)
    b_hh = inp["b_hh"].astype(np.float32)
    out_W = inp["out_W"].astype(np.float32)               # [V, 2048]
    out_b = inp["out_b"].astype(np.float32)

    bias_rz = col((b_ih[:4096] + b_hh[:4096]))            # [128, 32]
    bias_in = col(b_ih[4096:6144])                        # [128, 16]
    bias_hn = col(b_hh[4096:6144])

    base = {
        "emb_col": emb_col, "h0_col": h0_col,
        "attn_w": attn_w, "attn_b": inp["attn_b"].astype(np.float32)[None, :],
        "enc": enc_f,
        "bias_rz": bias_rz, "bias_in": bias_in, "bias_hn": bias_hn,
        "ones": np.ones((1, 1), np.float32),
    }

    # padded out_W / out_b
    P = N_CORES * VS
    out_W_pad = np.zeros((P, H), np.float32)
    out_W_pad[:V] = out_W
    out_b_pad = np.full((P,), NEG_BIG, np.float32)
    out_b_pad[:V] = out_b

    in_maps = []
    for k in range(N_CORES):
        m = dict(base)
        sl = slice(256 * k, 256 * (k + 1))
        m["h0sl_col"] = np.ascontiguousarray(h0[sl].reshape(2, 128).T)
        Rk = comb_W[sl]                                   # [256, 4096]
        # [kk, (c*32+g)*128+q] = Rk[128c+q, 128g+kk]
        m["comb_w"] = np.ascontiguousarray(
            Rk.reshape(2, 128, 32, 128).transpose(3, 0, 2, 1)).reshape(128, 32 * 256)
        m["comb_b"] = np.ascontiguousarray(
            inp["comb_b"].astype(np.float32)[sl].reshape(2, 128).T)
        # [kk, (c*2+g)*128+q] = Ck[128c+q, 128g+kk]
        Ck = W_ih[:, sl]                                  # [6144, 256]
        m["ih_w"] = np.ascontiguousarray(
            Ck.reshape(48, 128, 2, 128).transpose(3, 0, 2, 1)).reshape(128, 2 * 6144)
        Dk = W_hh[:, sl]
        m["hh_w"] = np.ascontiguousarray(
            Dk.reshape(48, 128, 2, 128).transpose(3, 0, 2, 1)).reshape(128, 2 * 6144)
        v0 = k * VS
        Mk = out_W_pad[v0:v0 + NT * 512]                  # [6144, 2048]
        m["out_w"] = np.ascontiguousarray(
            Mk.reshape(NT, 512, NKH, 128).transpose(0, 3, 2, 1)
        ).reshape(NT, 128, NKH * 512)
        Tk = out_W_pad[v0 + NT * 512:v0 + VS]             # [139, 2048]
        m["out_wt"] = np.ascontiguousarray(
            Tk.reshape(TAIL, NKH, 128).transpose(2, 1, 0)).reshape(128, NKH * TAIL)
        m["out_b"] = out_b_pad[None, v0:v0 + VS]
        in_maps.append(m)
    return in_maps


def kernel(**inputs):
    sys.path.insert(0, "/opt/trn_rl_repo")
    from concourse.bass_utils import run_bass_kernel_spmd

    if "nc" not in _CACHE:
        _CACHE["nc"] = _build()
    nc = _CACHE["nc"]

    in_maps = _prep_inputs(inputs)
    res = run_bass_kernel_spmd(nc, in_maps, core_ids=list(range(N_CORES)))
    r = res.results

    logp = np.concatenate([r[k]["logp"] for k in range(N_CORES)],
                          axis=1)[:, :V].astype(np.float32)
    h1 = r[0]["h1"].reshape(1, 1, H).astype(np.float32)
    attnw = r[0]["attnw"].astype(np.float32)
    return (logp, h1, attnw)


if __name__ == "__main__":
    sys.path.insert(0, os.path.dirname(os.path.abspath(__file__)))
    import reference
    inputs = {k: np.asarray(v) for k, v in reference.setup_inputs().items()}
    out = kernel(**inputs)
    ref = reference.reference(**inputs)
    for name, a, b in zip(("logp", "h1", "attnw"), out, ref):
        a, b = np.asarray(a), np.asarray(b)
        scale = max(np.abs(b).max(), 1e-30)
        print(f"{name}: max abs err {np.abs(a - b).max():.3e} "
              f"(rel {np.abs(a - b).max() / scale:.3e})")
